# revision 25
# baseline (speedup 1.0000x reference)
"""MultiHeadAttention (B=2, S=2048, D=1024, H=16, causal) on 8 trn2 NeuronCores.

Sharding: tensor-parallel over heads (2 heads/core) for QKV projections and
attention; two AllToAlls (one per batch) re-shard context rows so the output
projection is data-parallel over rows; bias added on device. Host only
slices/transposes/casts inputs and reassembles outputs.

Per-core output rows: global rows [c*256,(c+1)*256) (batch 0 part) and
[2048+c*256, 2048+(c+1)*256) (batch 1 part).

Pipeline layout (v2):
  - x is loaded per 512-row tile across 4 DMA queues so the first projection
    matmul can start as soon as row-tile 0 lands (~3us) instead of after the
    full 8 MB.
  - V is projected directly into [keys, head] layout (lhsT = x chunk,
    rhs = Wv chunk), eliminating the 64 SBUF xbar transposes of v1. The
    stationary V tiles are padded to 128 columns (col 64 = ones for the
    softmax denominator, cols 65..127 = 0) so FWL stays enabled.
  - Attention query-tiles are interleaved with projection / output-projection
    work on the PE queue ("filler units") so the exp latency on the ACT
    engine never starves the PE.
  - A2A collective + gather DMA live on the gpsimd queue; in v1 the gather
    trigger sat on the scalar queue and blocked ~40us of exp work behind the
    collective semaphore.
  - softmax denominators: reciprocal_approx_fast in f32 (then bitcast f32r
    for the selector-matmul partition broadcast); v1's f32r InstReciprocal
    was 3.35us per segment.

Math notes:
  - torch-Linear semantics: q = x @ Wq.T etc. Host passes transposed weight
    shards so q/k matmuls contract over the SBUF partition dim.
  - softmax without max-subtraction (scores*inv_scale is O(1), exp is safe);
    denominator comes from a ones-column appended to V in the attn@V matmul.
  - reference quirk preserved: scale = 1/(D**0.25).
"""

import sys
import types

import numpy as np
import ml_dtypes

N_CORES = 8
B, S, D = 2, 2048, 1024
H = 16
HEAD = 64
ROWS = B * S               # 4096
ROWS_PER_CORE = ROWS // N_CORES  # 512
INV_SCALE = 1.0 / (D ** 0.25)
QT = 512                   # query tile (free dim)
KT = 128                   # key tile (partition dim)
RH = 128                   # rows per core per A2A segment

BF16 = ml_dtypes.bfloat16
FP8 = ml_dtypes.float8_e4m3fn

_compiled = None


def _install_axon_profile_shim():
    """Provide antenv.axon_hooks (missing from this image) so trace=True works,
    and neuter the artifact upload (no bucket access in-container)."""
    if "antenv.axon_hooks" not in sys.modules:
        mod = types.ModuleType("antenv.axon_hooks")
        mod._hook = None
        mod.set_axon_ntff_profile_hook = lambda h: setattr(mod, "_hook", h)
        mod.get_axon_ntff_profile_hook = lambda: mod._hook
        sys.modules["antenv.axon_hooks"] = mod
        try:
            import antenv
            antenv.axon_hooks = mod
        except ImportError:
            pass
    mod = sys.modules["antenv.axon_hooks"]
    if mod._hook is None:
        try:
            from trn_agent_boot.trn_boot import _ntff_profile_via_ctypes
            mod.set_axon_ntff_profile_hook(
                _ntff_profile_via_ctypes("/opt/axon/libaxon_pjrt.so"))
        except Exception:
            pass
    try:
        import concourse.bass_utils as bu
        bu.upload_artifacts = lambda tmpdir: tmpdir
    except Exception:
        pass


def _build_program(debug=False):
    import concourse.bass as bass
    import concourse.bacc as bacc
    import concourse.mybir as mybir
    import concourse.tile as tile
    from concourse.bass import ts

    f32 = mybir.dt.float32
    f8 = mybir.dt.float8e4
    f32r = mybir.dt.float32r
    bf16 = mybir.dt.bfloat16
    Exp = mybir.ActivationFunctionType.Exp

    nc = bacc.Bacc(num_devices=N_CORES)
    dbg = {}
    if debug:
        dbg["qT"] = nc.dram_tensor("dbg_qT", [128, ROWS], bf16,
                                   kind="ExternalOutput")
        dbg["kT"] = nc.dram_tensor("dbg_kT", [128, ROWS], bf16,
                                   kind="ExternalOutput")
        dbg["v"] = nc.dram_tensor("dbg_v", [128, 32, 2, 128], bf16,
                                  kind="ExternalOutput")
        dbg["ctx0"] = nc.dram_tensor("dbg_ctx0", [64, ROWS], bf16,
                                     kind="ExternalOutput")
        dbg["ctx1"] = nc.dram_tensor("dbg_ctx1", [64, ROWS], bf16,
                                     kind="ExternalOutput")
        dbg["den"] = nc.dram_tensor("dbg_den", [4, 4, QT], f32,
                                    kind="ExternalOutput")
        dbg["rec"] = nc.dram_tensor("dbg_rec", [4, 4, QT], f32,
                                    kind="ExternalOutput")

    xT = nc.dram_tensor("xT", [D, ROWS], bf16, kind="ExternalInput")
    wqT = nc.dram_tensor("wqT", [128, 8, 128], bf16, kind="ExternalInput")
    wkT = nc.dram_tensor("wkT", [128, 8, 128], bf16, kind="ExternalInput")
    wvT = nc.dram_tensor("wvT", [128, 8, 128], bf16, kind="ExternalInput")
    woT = nc.dram_tensor("woT", [128, 8, D], bf16, kind="ExternalInput")
    bo = nc.dram_tensor("bo", [D], f32, kind="ExternalInput")
    masksq = nc.dram_tensor("masksq", [128, 128], bf16, kind="ExternalInput")
    sel = nc.dram_tensor("sel", [4, 4 * HEAD], f32r, kind="ExternalInput")
    out_rows = nc.dram_tensor("out_rows", [ROWS_PER_CORE, D], f32,
                              kind="ExternalOutput")

    with tile.TileContext(nc) as tc:
        with (
            tc.tile_pool(name="persist", bufs=1) as persist,
            tc.tile_pool(name="cp", bufs=4) as cp,
            tc.tile_pool(name="attn", bufs=6) as attn_pool,
            tc.tile_pool(name="nrm", bufs=4) as nrm_pool,
            tc.tile_pool(name="ps_work", bufs=2, space="PSUM") as ps_work,
            tc.tile_pool(name="ps_sc", bufs=2, space="PSUM") as ps_sc,
            tc.tile_pool(name="ps_av", bufs=2, space="PSUM") as av_pool,
            tc.tile_pool(name="dram", bufs=1, space="DRAM") as dram,
        ):
            # ---- persistent SBUF state ----
            # x row-tiles: one tile per 512 rows so deps are per-row-tile
            x_sb = [persist.tile([128, 8, QT], bf16, tag=f"x{rt}",
                                 name=f"x{rt}") for rt in range(8)]
            wq_sb = persist.tile([128, 8, 128], bf16)
            wk_sb = persist.tile([128, 8, 128], bf16)
            wv_sb = persist.tile([128, 8, 128], bf16)
            woT_sb = persist.tile([128, 8, D], bf16)          # 16 KB/part
            qT_sb = persist.tile([128, ROWS], bf16)           # 8 KB/part
            kT_sb = persist.tile([128, ROWS], bf16)
            # V stationary tiles [keys, (head, 64+pad)]; col 64 of each head
            # block is the ones column, 65..127 zero padding (keeps FWL on)
            v_sb = [persist.tile([128, 2, 128], bf16, tag=f"v{rt}",
                                 name=f"v{rt}") for rt in range(32)]
            ctx_sb = [persist.tile([64, ROWS], bf16, tag=f"ctx{h}",
                                   name=f"ctx{h}")
                      for h in range(2)]
            mask_sb = persist.tile([128, 128], bf16)
            sel_sb = persist.tile([4, 4 * HEAD], f32r)
            den_all = [persist.tile([4, QT], f32, tag=f"den{g}",
                                    name=f"den{g}") for g in range(5)]
            den_rec = [persist.tile([4, QT], f32r, tag=f"rec{g}",
                                    name=f"rec{g}") for g in range(5)]
            den_rcf = [persist.tile([4, QT], f32, tag=f"rcf{g}",
                                    name=f"rcf{g}") for g in range(5)]
            bo_sb = persist.tile([128, D], f32)
            a2a_sb = [persist.tile([128, 8, RH], bf16, tag=f"a2a{g}",
                                   name=f"a2a{g}") for g in range(4)]

            # ---- loads ----
            # The scalar queue carries ONLY wq/mask/sel: exp lives on the
            # scalar engine, and x-DMA triggers there block the exp stream
            # behind DMA-ring backpressure (v3: first exp at 45us). All x
            # bulk goes on sync+gpsimd, row-tile-major.
            nc.scalar.dma_start(wq_sb[:], wqT[:])
            nc.scalar.dma_start(mask_sb[:], masksq[:])
            nc.scalar.dma_start(sel_sb[:], sel[:])
            # rt0-2 also use the scalar queue (exp doesn't ramp until
            # ~20us); the rest stays off it
            qi = 0
            for rt in range(8):
                qs = [nc.sync, nc.gpsimd, nc.scalar] if rt < 3 else \
                     [nc.sync, nc.gpsimd]
                for kt in range(8):
                    qs[qi % len(qs)].dma_start(
                        x_sb[rt][:, kt, :],
                        xT[ts(kt, 128), ts(rt, QT)])
                    qi += 1
                if rt == 0:
                    nc.sync.dma_start(wk_sb[:], wkT[:])
                    nc.gpsimd.dma_start(wv_sb[:], wvT[:])
            # late-needed tensors (trigger-only cost, used from ~80us on)
            nc.sync.dma_start(woT_sb[:], woT[:])
            nc.sync.dma_start(
                bo_sb[:], bass.AP(tensor=bo, offset=0,
                                  ap=[[0, 128], [1, D]]))
            # V tile ones column + zero padding
            for rt in range(32):
                nc.gpsimd.memset(v_sb[rt][:, :, 64:65], 1.0)
                nc.vector.memset(v_sb[rt][:, :, 65:128], 0.0)

            # ---- projection units ----
            def pair_qk(jobs):
                """Two q/k projections as 4 filler units; the two 8-deep
                accumulation chains interleave so consecutive matmuls hit
                different psum banks (and share the stationary when both
                jobs use the same weight). jobs = [(w_sb, rt, dst), x2].
                The 4 units MUST be popped consecutively (shared psums)."""
                (wA, rtA, dstA), (wB, rtB, dstB) = jobs
                cell = {}

                def mk(i):
                    def f():
                        if i == 0:
                            pqa = ps_work.tile([128, QT], f32, tag="work")
                            pqb = ps_work.tile([128, QT], f32, tag="work")
                            cell["a"], cell["b"] = pqa, pqb
                        pa, pb = cell["a"], cell["b"]
                        for kt in (2 * i, 2 * i + 1):
                            nc.tensor.matmul(pa, wA[:, kt, :],
                                             x_sb[rtA][:, kt, :],
                                             start=(kt == 0), stop=(kt == 7))
                            nc.tensor.matmul(pb, wB[:, kt, :],
                                             x_sb[rtB][:, kt, :],
                                             start=(kt == 0), stop=(kt == 7))
                        if i == 3:
                            nc.vector.tensor_copy(dstA[:, ts(rtA, QT)], pa)
                            nc.vector.tensor_copy(dstB[:, ts(rtB, QT)], pb)
                    return f

                return [mk(i) for i in range(4)]

            def proj_q(rt, w_sb=None, dst=None):
                w_sb = wq_sb if w_sb is None else w_sb
                dst = qT_sb if dst is None else dst
                ps = ps_work.tile([128, QT], f32, tag="work")
                for kt in range(8):
                    nc.tensor.matmul(ps, w_sb[:, kt, :], x_sb[rt][:, kt, :],
                                     start=(kt == 0), stop=(kt == 7))
                nc.vector.tensor_copy(dst[:, ts(rt, QT)], ps)

            def proj_k(rt):
                proj_q(rt, wk_sb, kT_sb)

            def proj_v(rt, quarters=(0, 1, 2, 3)):
                # one 128-row key tile per call, direct [keys, head] layout
                for i4 in quarters:
                    rt128 = rt * 4 + i4
                    ps = ps_work.tile([128, QT], f32, tag="work")
                    for kt in range(8):
                        nc.tensor.matmul(
                            ps[:, 0:128],
                            x_sb[rt][:, kt, ts(i4, 128)],
                            wv_sb[:, kt, :],
                            start=(kt == 0), stop=(kt == 7))
                    nc.vector.tensor_copy(
                        v_sb[rt128][:, :, 0:64],
                        ps[:, 0:128].rearrange("p (h m) -> p h m", h=2))

            def pair_op(g):
                """Both 512-col halves of segment g's output projection as
                4 filler units; chains alternate banks and share the
                stationary a2a tile. Units must be popped consecutively."""
                cell = {}

                def mk(i):
                    def f():
                        if i == 0:
                            poa = ps_work.tile([128, QT], f32, tag="work")
                            pob = ps_work.tile([128, QT], f32, tag="work")
                            cell["a"], cell["b"] = poa, pob
                        pa, pb = cell["a"], cell["b"]
                        for t in (2 * i, 2 * i + 1):
                            nc.tensor.matmul(pa, a2a_sb[g][:, t, :],
                                             woT_sb[:, t, 0:QT],
                                             start=(t == 0), stop=(t == 7))
                            nc.tensor.matmul(pb, a2a_sb[g][:, t, :],
                                             woT_sb[:, t, QT:D],
                                             start=(t == 0), stop=(t == 7))
                        if i == 3:
                            for nh, ps in ((0, pa), (1, pb)):
                                ob = cp.tile([128, QT], f32, tag="ob")
                                nc.vector.tensor_add(ob, ps,
                                                     bo_sb[:, ts(nh, QT)])
                                nc.sync.dma_start(
                                    out_rows[ts(g, 128), ts(nh, QT)], ob)
                    return f

                return [mk(i) for i in range(4)]

            # ---- attention ----
            def attn_qt(b, qt, g, u_base, fillers, den_idx=None):
                """One 512-query tile; pops one filler unit per jk tile so the
                PE never waits on the exp pipeline."""
                n_k = 4 * qt + 4
                q0 = b * S + qt * QT
                ps_av = [av_pool.tile([128, QT], f32, tag="av",
                                      name=f"av{b}_{qt}_{h}")
                         for h in range(2)]
                def av(jk, at, c0):
                    for h in range(2):
                        nc.tensor.matmul(
                            ps_av[h][:, c0:QT],
                            v_sb[b * 16 + jk][:, h, :],
                            at[:, h, c0:QT],
                            start=(jk == 0), stop=(jk == n_k - 1))

                prev = None  # software pipeline: av lags one jk behind sc
                for jk in range(n_k):
                    o = jk - 4 * qt       # >=0 on the diagonal band
                    c0 = max(o, 0) * 128  # first live query column
                    k0 = b * S + jk * KT
                    ps_s = ps_sc.tile([128, 2, QT], f32, tag="sc")
                    at = attn_pool.tile([128, 2, QT], bf16, tag="at")
                    for h in range(2):
                        hs = slice(h * HEAD, (h + 1) * HEAD)
                        nc.tensor.matmul(
                            ps_s[:, h, c0:QT],
                            kT_sb[hs, k0:k0 + KT],
                            qT_sb[hs, q0 + c0:q0 + QT],
                            start=True, stop=True)
                    nc.scalar.activation(at[:, :, c0:QT], ps_s[:, :, c0:QT],
                                         Exp, scale=INV_SCALE)
                    if o >= 0:
                        # partial causal sub-block: cols [c0, c0+128)
                        nc.vector.tensor_mul(
                            at[:, :, c0:c0 + 128],
                            at[:, :, c0:c0 + 128],
                            mask_sb[:, None, :].to_broadcast([128, 2, 128]))
                    if fillers:
                        fu = fillers.pop(0)  # PE filler while exp runs
                        if fu is not None:
                            fu()
                    if prev is not None:
                        av(*prev)
                    prev = (jk, at, c0)
                av(*prev)
                # drain: ctx rows + denominator row
                for h in range(2):
                    u = u_base + h
                    nc.vector.tensor_copy(ctx_sb[h][:, q0:q0 + QT],
                                          ps_av[h][0:HEAD, :])
                    dt = nrm_pool.tile([HEAD + 1, QT], f32, tag="dtmp")
                    nc.vector.tensor_copy(dt[HEAD:HEAD + 1, :],
                                          ps_av[h][HEAD:HEAD + 1, :])
                    di = g if den_idx is None else den_idx
                    nc.sync.dma_start(den_all[di][u:u + 1, :],
                                      dt[HEAD:HEAD + 1, :])

            def norm_seg(b, qts, di):
                """recip + selector-broadcast + ctx multiply for query tiles
                qts; den rows start at partition 0 of den_all[di]."""
                nu = 2 * len(qts)
                nc.vector.reciprocal_approx_fast(den_rcf[di][0:nu, :],
                                                 den_all[di][0:nu, :])
                with nc.allow_low_precision(
                        reason="softmax denominators: f32r keeps ~19 "
                               "mantissa bits, ample for a 1/x broadcast"):
                    nc.vector.tensor_copy(den_rec[di][0:nu, :],
                                          den_rcf[di][0:nu, :])
                for u in range(nu):
                    h = u % 2
                    qt = qts[u // 2]
                    q0 = b * S + qt * QT
                    ps_b = ps_work.tile([128, QT], f32, tag="work")
                    nc.tensor.matmul(ps_b[0:HEAD, :],
                                     sel_sb[0:nu, ts(u, HEAD)],
                                     den_rec[di][0:nu, :],
                                     start=True, stop=True)
                    nc.vector.tensor_mul(ctx_sb[h][:, q0:q0 + QT],
                                         ctx_sb[h][:, q0:q0 + QT],
                                         ps_b[0:HEAD, :])

            def a2a_seg(b, half, qts, tag):
                """Stage normalized ctx rows for qts and run the AllToAll;
                gather is deferred one collective (gpsimd queue ordering)."""
                g = b * 2 + half
                r0 = b * S + qts[0] * QT
                nrows = len(qts) * QT
                rh = nrows // 8
                a2a_in = dram.tile([8, 128, rh], bf16, tag=f"a2ain{tag}",
                                   name=f"a2ain{tag}")
                a2a_out = dram.tile([8, 128, rh], bf16, tag=f"a2aout{tag}",
                                    name=f"a2aout{tag}")
                for h in range(2):
                    nc.sync.dma_start(
                        a2a_in[:, h * 64:(h + 1) * 64, :]
                        .rearrange("s p r -> p s r"),
                        ctx_sb[h][:, r0:r0 + nrows]
                        .rearrange("p (s r) -> p s r", s=8))
                nc.gpsimd.collective_compute(
                    "AllToAll", mybir.AluOpType.bypass,
                    replica_groups=[list(range(N_CORES))],
                    ins=[a2a_in[:].opt()], outs=[a2a_out[:].opt()])
                dst = a2a_sb[g]
                roff = 0 if qts[0] % 2 == 0 else RH - rh
                pending_gathers.append(lambda: nc.gpsimd.dma_start(
                    dst[:, :, roff:roff + rh],
                    a2a_out[:].rearrange("t p r -> p t r")))
                while len(pending_gathers) > 1:
                    pending_gathers.pop(0)()

            def norm_and_a2a(b, half, qts=None, tag=None, den_idx=None):
                g = b * 2 + half
                if qts is None:
                    qts = (2 * half, 2 * half + 1)
                norm_seg(b, qts, g if den_idx is None else den_idx)
                a2a_seg(b, half, qts, tag if tag is not None else str(g))

            def outproj_pair(g, r0=0, r1=RH):
                # both column halves interleaved, full row range [r0, r1)
                nr = r1 - r0
                pa = ps_work.tile([128, QT], f32, tag="work")
                pb = ps_work.tile([128, QT], f32, tag="work")
                for t in range(8):
                    nc.tensor.matmul(pa[0:nr, :], a2a_sb[g][:, t, r0:r1],
                                     woT_sb[:, t, 0:QT],
                                     start=(t == 0), stop=(t == 7))
                    nc.tensor.matmul(pb[0:nr, :], a2a_sb[g][:, t, r0:r1],
                                     woT_sb[:, t, QT:D],
                                     start=(t == 0), stop=(t == 7))
                for nh, ps in ((0, pa), (1, pb)):
                    ob = cp.tile([128, QT], f32, tag="ob")
                    nc.vector.tensor_add(ob[0:nr, :], ps[0:nr, :],
                                         bo_sb[0:nr, ts(nh, QT)])
                    nc.sync.dma_start(
                        out_rows[g * RH + r0:g * RH + r1, ts(nh, QT)],
                        ob[0:nr, :])

            # ---- pipeline ----
            # Row-tile 0 up front; all other projections + early output
            # projections ride as ~0.5-1us PE filler units through the
            # attention jk-loops. Paired units (pair_qk/pair_op) interleave
            # two accumulation chains across psum banks and must be popped
            # consecutively. Density keeps every core HAM-warm and the cores
            # in lockstep (short A2A barriers).
            pending_gathers = []
            proj_q(0)
            proj_k(0)
            attn_qt(0, 0, g=0, u_base=0, fillers=(
                [lambda i=i: proj_v(0, (i,)) for i in range(4)]))
            for fu in pair_qk([(wq_sb, 1, qT_sb), (wk_sb, 1, kT_sb)]):
                fu()
            attn_qt(0, 1, g=0, u_base=2, fillers=(
                [lambda i=i: proj_v(1, (i,)) for i in range(4)]
                + pair_qk([(wq_sb, 2, qT_sb), (wq_sb, 3, qT_sb)])))
            norm_and_a2a(0, 0)
            attn_qt(0, 2, g=1, u_base=0, fillers=(
                pair_qk([(wk_sb, 2, kT_sb), (wk_sb, 3, kT_sb)])
                + [lambda i=i: proj_v(2, (i,)) for i in range(4)]
                + pair_qk([(wq_sb, 4, qT_sb), (wq_sb, 5, qT_sb)])))
            attn_qt(0, 3, g=1, u_base=2, fillers=(
                [lambda i=i: proj_v(3, (i,)) for i in range(4)]
                + pair_qk([(wk_sb, 4, kT_sb), (wk_sb, 5, kT_sb)])
                + [lambda i=i: proj_v(4, (i,)) for i in range(4)]
                + [None, None, None, None]))
            norm_and_a2a(0, 1)
            attn_qt(1, 0, g=2, u_base=0, fillers=(
                pair_qk([(wq_sb, 6, qT_sb), (wq_sb, 7, qT_sb)])))
            attn_qt(1, 1, g=2, u_base=2, fillers=(
                [lambda i=i: proj_v(5, (i,)) for i in range(4)]
                + pair_qk([(wk_sb, 6, kT_sb), (wk_sb, 7, kT_sb)])))
            norm_and_a2a(1, 0)
            attn_qt(1, 2, g=3, u_base=0, fillers=(
                [lambda i=i: proj_v(6, (i,)) for i in range(4)]
                + pair_op(0)
                + [None, None, None, None]))
            norm_seg(1, (2,), 3)  # qt2 norm off the tail critical path
            attn_qt(1, 3, g=3, u_base=0, den_idx=4, fillers=(
                [lambda i=i: proj_v(7, (i,)) for i in range(4)]
                + [None, None]
                + pair_op(1)
                + [None, None]))
            norm_seg(1, (3,), 4)
            a2a_seg(1, 1, (2, 3), "3")
            while pending_gathers:
                pending_gathers.pop(0)()
            pair_units = pair_op(2)
            for u in pair_units:
                u()
            outproj_pair(3)

            if debug:
                nc.sync.dma_start(dbg["qT"][:], qT_sb[:])
                nc.sync.dma_start(dbg["kT"][:], kT_sb[:])
                for rt in range(32):
                    nc.sync.dma_start(dbg["v"][:, rt, :, :], v_sb[rt][:])
                nc.sync.dma_start(dbg["ctx0"][:], ctx_sb[0][:])
                nc.sync.dma_start(dbg["ctx1"][:], ctx_sb[1][:])
                for g in range(4):
                    nc.sync.dma_start(dbg["den"][g], den_all[g][:])
                    nc.sync.dma_start(dbg["rec"][g],
                                      den_rec[g][:].bitcast(f32))

    nc.finalize()
    return nc


def _make_masksq():
    p = np.arange(128)[:, None]
    r = np.arange(128)[None, :]
    return (p <= r).astype(BF16)


def _make_sel():
    # sel[k, u*64+m] = 1 if k == u : broadcasts den_rec row u over 64 partitions
    s = np.zeros((4, 4 * HEAD), np.float32)
    for u in range(4):
        s[u, u * HEAD:(u + 1) * HEAD] = 1.0
    return s


def _wlayout(wT, dt=None):
    # [1024, m] -> [128, 8, m] with dst[p, t, :] = wT[t*128+p, :]
    m = wT.shape[1]
    return np.ascontiguousarray(
        wT.reshape(8, 128, m).transpose(1, 0, 2)).astype(
            BF16 if dt is None else dt)


def _shard_inputs(x, Wq, Wk, Wv, Wo, bo):
    xT = np.ascontiguousarray(
        x.reshape(ROWS, D).T).astype(BF16)            # [D, 4096]
    woT = _wlayout(Wo.T)                              # [128, 8, D]
    masksq = _make_masksq()
    sel = _make_sel()
    bo32 = np.ascontiguousarray(bo.astype(np.float32))
    maps = []
    for c in range(N_CORES):
        rs = slice(c * 128, (c + 1) * 128)
        maps.append({
            "xT": xT,
            "wqT": _wlayout(Wq[rs].T),
            "wkT": _wlayout(Wk[rs].T),
            "wvT": _wlayout(Wv[rs].T),
            "woT": woT,
            "bo": bo32,
            "masksq": masksq,
            "sel": sel,
        })
    return maps


def kernel(x, Wq, Wk, Wv, Wo, bo, trace=False):
    global _compiled
    _install_axon_profile_shim()
    from concourse.bass_utils import run_bass_kernel_spmd

    x = np.asarray(x, dtype=np.float32)
    Wq = np.asarray(Wq, dtype=np.float32)
    Wk = np.asarray(Wk, dtype=np.float32)
    Wv = np.asarray(Wv, dtype=np.float32)
    Wo = np.asarray(Wo, dtype=np.float32)
    bo = np.asarray(bo, dtype=np.float32)

    if _compiled is None:
        _compiled = _build_program()
    nc = _compiled

    in_maps = _shard_inputs(x, Wq, Wk, Wv, Wo, bo)
    res = run_bass_kernel_spmd(nc, in_maps, core_ids=list(range(N_CORES)),
                               trace=trace)
    out = np.empty((ROWS, D), np.float32)
    for c in range(N_CORES):
        r = res.results[c]["out_rows"]
        for g in range(4):
            b, half = g // 2, g % 2
            r0 = b * S + half * (S // 2) + c * RH
            out[r0:r0 + RH] = r[g * RH:(g + 1) * RH]
    out = out.reshape(B, S, D)
    if trace:
        kernel.last_exec_time_ns = res.exec_time_ns
        kernel.last_results = res
    return out
